# revision 24
# baseline (speedup 1.0000x reference)
"""A-Connect conv kernel for TRN2, data-parallel over batch on 8 NeuronCores.

Computation (per sample b):
    Z[b] = conv2d(X[b], W * Werr[b], SAME) + bias * Berr[b]; out = relu(Z)

Mapping: batch 32 -> 4 samples per core. The 3x3 conv is decomposed with
1D Winograd F(4,3) along the width axis (points {0, +-1, +-2, inf}), which
cuts tensor-engine work to 1/2 of direct convolution: for each Winograd
position u in 0..5 the device computes

    m_u[f, (y, tx)] = sum_dy sum_cin U[u, dy, cin, f] * V_u[cin, y+dy, tx]

as 3 PSUM-accumulated matmuls with N=512 moving streams. The V plane per u
is stored as two row-halves (rows 0..33 / 32..65 of the 66 padded rows) so
that every (u, chunk, dy) moving operand is one contiguous 512-col run and
the first half-plane can land before the rest (short critical head).
Weight-stationary slices are shared between the two row chunks (the c0/c1
matmuls of one (u, dy) are issued back to back under one LDWEIGHTS).

The Winograd input transform V = B^T d and weight transform U = G g are
linear data prep on the host; the output transform z = A^T m, bias and
relu also run on the host. The device ships the six raw m_u tiles per
(sample, F-half) as bf16, evacuated from PSUM by DVE (chunk 0) and ACT
(chunk 1) in parallel, staged in SBUF and written back as two 786 KB
DMAs per (sample, F-half) so the writes run at near line rate.

Queue discipline: sample 0's inputs ride the two HWDGE queues (sync +
scalar) split into half-plane pieces; samples 1..3 prefetch on the gpsimd
SWDGE queue one sample ahead, head-blocked by a tiny copy dependency on
sample 0's last tile so the prefetch stream cannot steal HBM bandwidth
from the latency-critical head. A burst of dependency-free warmup matmuls
releases the PE HAM clock gate while the first input DMAs are in flight.
Measured rel err vs the fp32 reference: ~9.5e-3 (bf16 V/U/m storage).
"""

import numpy as np
import ml_dtypes

B, H, Wd, Cin, F, KH, KW = 32, 64, 64, 128, 256, 3, 3
NCORES = 8
BPC = B // NCORES  # samples per core
NU = 6  # Winograd F(4,3) positions
NDY = 3  # row taps
NTX = Wd // 4  # 16 column tiles (4 output cols each)
NC = 2  # row chunks (32 output rows each)
NFH = 2  # halves of F on the 128 output partitions
HCOLS = 34 * NTX  # 544 cols per (u, half): 34 V-rows of 16 tx
VLEN = NC * NU * HCOLS  # 6528
NSHIP = 5  # tiles shipped per (fh, chunk): dneg, qneg, m5, z2, z0
SCOLS = NSHIP * NC * 512  # 5120 staging cols per (sample, fh)

_compiled = None  # cached Bass program so repeated kernel() calls reuse it


def _build_bass():
    from concourse import bacc, tile, mybir

    nc = bacc.Bacc("TRN2", target_bir_lowering=False, debug=False)
    bf16 = mybir.dt.bfloat16
    f32 = mybir.dt.float32

    vts = nc.dram_tensor("vts", [BPC, Cin, VLEN], bf16, kind="ExternalInput")
    wu = nc.dram_tensor("wu", [BPC, Cin, NFH * NU * NDY, 128], bf16, kind="ExternalInput")
    y = nc.dram_tensor("y", [BPC, NFH, 128, SCOLS], bf16, kind="ExternalOutput")

    with tile.TileContext(nc) as tc:
        with (
            tc.tile_pool(name="vpool", bufs=4) as vpool,
            tc.tile_pool(name="wpool", bufs=4) as wpool,
            tc.tile_pool(name="spool", bufs=4) as spool,
            tc.tile_pool(name="tpool", bufs=3) as tpool,
            tc.tile_pool(name="cpool", bufs=1) as cpool,
            tc.tile_pool(name="pspool", bufs=6, space="PSUM") as pspool,
            tc.tile_pool(name="wupool", bufs=1, space="PSUM") as wupool,
        ):
            # PE warmup: dependency-free matmuls release the HAM clock
            # gate (~3.4us of sustained activity) while the first input
            # DMAs land.
            # PE warmup: dependency-free matmuls release the HAM clock
            # gate (~3.4us of sustained activity) while the first input
            # DMAs land, so the real stream starts at 2.4GHz.
            wu_in = cpool.tile([128, 512], bf16)
            nc.vector.memset(wu_in[:], 0.0)
            wu_ps = wupool.tile([128, 512], f32)
            NWARM = 14
            for i in range(NWARM):
                nc.tensor.matmul(
                    wu_ps[:],
                    wu_in[:, :128],
                    wu_in[:],
                    start=(i == 0),
                    stop=(i == NWARM - 1),
                )

            vt = [None] * BPC
            wt = [None] * BPC

            def prefetch(b):
                wt[b] = wpool.tile([Cin, NFH * NU * NDY, 128], bf16, name="wt")
                vt[b] = vpool.tile([Cin, VLEN], bf16, name="vt")
                if b == 0:
                    # critical head, balanced across the two HWDGE
                    # queues so every piece lands just before its first
                    # use. half0 is split at u3 so the very first
                    # matmuls (fh0, c0, u0) only wait for 418+590 KB.
                    H3 = 3 * HCOLS
                    nc.sync.dma_start(vt[b][:, 0:H3], vts[b][:, 0:H3])
                    nc.scalar.dma_start(wt[b][:, 0 : NU * NDY, :], wu[b][:, 0 : NU * NDY])
                    nc.sync.dma_start(
                        vt[b][:, H3 : NU * HCOLS], vts[b][:, H3 : NU * HCOLS]
                    )
                    nc.scalar.dma_start(
                        vt[b][:, NU * HCOLS : VLEN], vts[b][:, NU * HCOLS : VLEN]
                    )
                    nc.sync.dma_start(
                        wt[b][:, NU * NDY : NFH * NU * NDY, :],
                        wu[b][:, NU * NDY : NFH * NU * NDY],
                    )
                else:
                    # Prefetch in four half-pieces per sample, each
                    # preceded by a write-after-write gate copy into its
                    # destination. Every gate reads the SAME source (the
                    # last col of sample 0's wu-fh0 block, the earliest
                    # head piece to land): identical deps make the Tile
                    # scheduler keep program order, so the FIFO SWDGE
                    # ring drains exactly in first-use order and each
                    # sample's first-needed pieces arrive earliest —
                    # while sample 0's critical head loads still win the
                    # HBM bandwidth race against the prefetch stream.
                    gsrc = wt[0][:, NU * NDY - 1, :1]
                    H6 = NU * HCOLS
                    S18 = NU * NDY
                    nc.gpsimd.tensor_copy(vt[b][:, :1], gsrc)
                    nc.gpsimd.dma_start(vt[b][:, 0:H6], vts[b][:, 0:H6])
                    nc.gpsimd.tensor_copy(wt[b][:, 0, :1], gsrc)
                    nc.gpsimd.dma_start(wt[b][:, 0:S18, :], wu[b][:, 0:S18])
                    nc.gpsimd.tensor_copy(vt[b][:, H6 : H6 + 1], gsrc)
                    nc.gpsimd.dma_start(vt[b][:, H6:VLEN], vts[b][:, H6:VLEN])
                    nc.gpsimd.tensor_copy(wt[b][:, S18, :1], gsrc)
                    nc.gpsimd.dma_start(
                        wt[b][:, S18 : 2 * S18, :], wu[b][:, S18 : 2 * S18]
                    )

            add = mybir.AluOpType.add
            sub = mybir.AluOpType.subtract
            Copy = mybir.ActivationFunctionType.Copy
            # u visit order: fold sources land before their consumers
            # (a1<-u1, a3<-u3, s/dneg<-u2, p/qneg/p4/z2<-u4, m5<-u5,
            # t0/z0<-u0), so PSUM drains eagerly and the DVE/ACT/GPSIMD
            # fold chain pipelines under the matmul stream.
            U_ORDER = (1, 3, 2, 4, 5, 0)

            def emit_fh(b, fh, passes, stage, tail=False):
                # Ship 5 tiles per (fh, chunk) instead of the 6 raw m_u:
                #   dneg = m2-m1, qneg = m4-m3, m5, z2 = m1+m2+4(m3+m4),
                #   z0 = m0+..+m4 — the host recovers z1/z3 from
                #   dneg/qneg/m5. Cuts write-back traffic by 1/6.
                for ci, chunks in enumerate(passes):
                    final = ci == len(passes) - 1
                    a1, a3, s, p, p4, t0 = {}, {}, {}, {}, {}, {}
                    for u in U_ORDER:
                        pss = {}
                        for c in chunks:
                            pss[c] = pspool.tile([128, 512], f32, name="ps")
                        for dy in range(NDY):
                            ws = wt[b][:, (fh * NU + u) * NDY + dy, :]
                            for c in chunks:
                                base = c * NU * HCOLS + u * HCOLS + dy * NTX
                                nc.tensor.matmul(
                                    pss[c],
                                    ws,
                                    vt[b][:, base : base + 512],
                                    start=(dy == 0),
                                    stop=(dy == NDY - 1),
                                )
                        for c in chunks:
                            ps = pss[c]

                            def slot(j, c=c):
                                o = (j * NC + c) * 512
                                return stage[:, o : o + 512]

                            if u == 1:
                                a1[c] = tpool.tile([128, 512], f32, name="a1")
                                nc.scalar.activation(a1[c][:], ps[:], Copy)
                            elif u == 3:
                                a3[c] = tpool.tile([128, 512], f32, name="a3")
                                nc.scalar.activation(a3[c][:], ps[:], Copy)
                            elif u == 2:
                                s[c] = tpool.tile([128, 512], f32, name="s")
                                nc.vector.tensor_tensor(s[c][:], ps[:], a1[c][:], add)
                                nc.vector.tensor_tensor(slot(0), ps[:], a1[c][:], sub)
                            elif u == 4:
                                p[c] = tpool.tile([128, 512], f32, name="p")
                                nc.vector.tensor_tensor(p[c][:], ps[:], a3[c][:], add)
                                nc.vector.tensor_tensor(slot(1), ps[:], a3[c][:], sub)
                                p4[c] = tpool.tile([128, 512], f32, name="p4")
                                nc.scalar.activation(
                                    p4[c][:], p[c][:], Copy, scale=4.0
                                )
                                nc.gpsimd.tensor_tensor(
                                    slot(3), s[c][:], p4[c][:], add
                                )
                            elif u == 5:
                                nc.scalar.activation(slot(2), ps[:], Copy)
                            else:  # u == 0
                                t0[c] = tpool.tile([128, 512], f32, name="t0")
                                nc.vector.tensor_tensor(t0[c][:], ps[:], s[c][:], add)
                                nc.gpsimd.tensor_tensor(
                                    slot(4), t0[c][:], p[c][:], add
                                )
                        if final and u == 4:
                            nc.sync.dma_start(
                                y[b, fh, :, 0 : 4 * 512], stage[:, 0 : 4 * 512]
                            )
                        if final and u == 5:
                            nc.scalar.dma_start(
                                y[b, fh, :, 4 * 512 : 8 * 512],
                                stage[:, 4 * 512 : 8 * 512],
                            )
                        if final and u == 0:
                            qe = nc.sync if fh == 0 else nc.scalar
                            qe.dma_start(
                                y[b, fh, :, 8 * 512 : SCOLS],
                                stage[:, 8 * 512 : SCOLS],
                            )

            # deep prefetch: queue every sample's loads upfront; the
            # SWDGE ring drains them FIFO as bandwidth allows
            for b in range(BPC):
                prefetch(b)
            for b in range(BPC):
                if b == 0:
                    # fh0 runs chunk 0 alone first so compute starts as
                    # soon as half-plane 0 + fh0 weights have landed
                    plans = [(0, [[0], [1]]), (1, [[0, 1]])]
                else:
                    plans = [(0, [[0, 1]]), (1, [[0, 1]])]
                for fh, passes in plans:
                    stage = spool.tile([128, SCOLS], bf16, name="st")
                    emit_fh(b, fh, passes, stage, tail=(b == BPC - 1 and fh == 1))
    nc.compile()
    return nc


_AT = np.array(
    [
        [1, 1, 1, 1, 1, 0],
        [0, 1, -1, 2, -2, 0],
        [0, 1, 1, 4, 4, 0],
        [0, 1, -1, 8, -8, 1],
    ],
    np.float32,
)
_G = np.array(
    [
        [1 / 4, 0, 0],
        [-1 / 6, -1 / 6, -1 / 6],
        [-1 / 6, 1 / 6, -1 / 6],
        [1 / 24, 1 / 12, 1 / 6],
        [1 / 24, -1 / 12, 1 / 6],
        [0, 0, 1],
    ],
    np.float32,
)
_BT = np.array(
    [
        [4, 0, -5, 0, 1, 0],
        [0, -4, -4, 1, 1, 0],
        [0, 4, -4, -1, 1, 0],
        [0, -2, -1, 2, 1, 0],
        [0, 2, -1, -2, 1, 0],
        [0, 4, 0, -5, 0, 1],
    ],
    np.float32,
)


def _prep_inputs(X, W, bias, Werr, Berr):
    bf16 = ml_dtypes.bfloat16
    X, W, Werr = (np.asarray(a) for a in (X, W, Werr))
    # weight transform along dx: U[u] = G @ g on the per-sample perturbed
    # kernels, laid out [Cin, fh*18 + u*3 + dy, 128] for stationary slices
    memW = W[None] * Werr  # [B, dy, dx, Cin, F]
    U = np.einsum("ux,byxcf->bcuyf", _G, memW)  # [B, Cin, u, dy, F]
    wu = (
        U.reshape(B, Cin, NU, NDY, NFH, 128)
        .transpose(0, 1, 4, 2, 3, 5)
        .reshape(B, Cin, NFH * NU * NDY, 128)
    )
    wu = np.ascontiguousarray(wu, dtype=bf16)
    # input transform: V_u = B^T d over 6-col windows stride 4, on the
    # zero-padded input (rows/cols -1..64). Row-halves 0..33 / 32..65.
    Xp = np.zeros((B, Cin, H + 2, Wd + 2), np.float32)
    Xp[:, :, 1 : H + 1, 1 : Wd + 1] = X.transpose(0, 3, 1, 2)
    cols = np.arange(NTX) * 4
    d = np.stack([Xp[:, :, :, j + cols] for j in range(6)], axis=3)
    V = np.einsum("uj,bcyjt->bcuyt", _BT, d)  # [B, Cin, 6u, 66, 16]
    halves = np.stack([V[:, :, :, 0:34], V[:, :, :, 32:66]], axis=2)
    vts = np.ascontiguousarray(halves.reshape(B, Cin, VLEN), dtype=bf16)
    return vts, wu


def _postprocess(y_cores, bias, Berr):
    m = np.concatenate(y_cores, axis=0).astype(np.float32)  # [B, fh, 128, SCOLS]
    m = m.reshape(B, NFH, 128, NSHIP, NC, 32, NTX)
    dneg = m[:, :, :, 0]
    qneg = m[:, :, :, 1]
    m5 = m[:, :, :, 2]
    z2 = m[:, :, :, 3]
    z0 = m[:, :, :, 4]
    z1 = -(dneg + 2.0 * qneg)
    z3 = m5 - dneg - 8.0 * qneg
    z = np.stack([z0, z1, z2, z3])  # [4, B, fh, 128, c, y, t]
    # -> [B, (c,y)=64, (t,k)=64, (fh,p)=256]
    out = z.transpose(1, 4, 5, 6, 0, 2, 3).reshape(B, H, Wd, F)
    out = np.ascontiguousarray(out)
    out += (np.asarray(bias)[None] * np.asarray(Berr))[:, None, None, :]
    np.maximum(out, 0.0, out=out)
    return out


def kernel(X, W, bias, Werr, Berr):
    global _compiled
    from concourse.bass_utils import run_bass_kernel_spmd

    if _compiled is None:
        _compiled = _build_bass()
    nc = _compiled

    vts, wu = _prep_inputs(X, W, bias, Werr, Berr)
    in_maps = [
        {
            "vts": vts[c * BPC : (c + 1) * BPC],
            "wu": wu[c * BPC : (c + 1) * BPC],
        }
        for c in range(NCORES)
    ]
    res = run_bass_kernel_spmd(nc, in_maps, core_ids=list(range(NCORES)))
    return _postprocess([r["y"] for r in res.results], bias, Berr)


# revision 26
# speedup vs baseline: 1.1196x; 1.1196x over previous
"""A-Connect conv kernel for TRN2, data-parallel over batch on 8 NeuronCores.

Computation (per sample b):
    Z[b] = conv2d(X[b], W * Werr[b], SAME) + bias * Berr[b]; out = relu(Z)

Mapping: batch 32 -> 4 samples per core. The 3x3 conv is decomposed with
1D Winograd F(4,3) along the width axis (points {0, +-1, +-2, inf}), which
cuts tensor-engine work to 1/2 of direct convolution: for each Winograd
position u in 0..5 the device computes

    m_u[f, (y, tx)] = sum_dy sum_cin U[u, dy, cin, f] * V_u[cin, y+dy, tx]

as 3 PSUM-accumulated matmuls with N=512 moving streams. The V plane per u
is stored as two row-halves (rows 0..33 / 32..65 of the 66 padded rows) so
that every (u, chunk, dy) moving operand is one contiguous 512-col run and
the first half-plane can land before the rest (short critical head).
Weight-stationary slices are shared between the two row chunks (the c0/c1
matmuls of one (u, dy) are issued back to back under one LDWEIGHTS).

The Winograd input transform V = B^T d and weight transform U = G g are
linear data prep on the host; the output transform z = A^T m, bias and
relu also run on the host. The device ships the six raw m_u tiles per
(sample, F-half) as bf16, evacuated from PSUM by DVE (chunk 0) and ACT
(chunk 1) in parallel, staged in SBUF and written back as two 786 KB
DMAs per (sample, F-half) so the writes run at near line rate.

Queue discipline: sample 0's inputs ride the two HWDGE queues (sync +
scalar) split into half-plane pieces; samples 1..3 prefetch on the gpsimd
SWDGE queue one sample ahead, head-blocked by a tiny copy dependency on
sample 0's last tile so the prefetch stream cannot steal HBM bandwidth
from the latency-critical head. A burst of dependency-free warmup matmuls
releases the PE HAM clock gate while the first input DMAs are in flight.
Measured rel err vs the fp32 reference: ~9.5e-3 (bf16 V/U/m storage).
"""

import numpy as np
import ml_dtypes

B, H, Wd, Cin, F, KH, KW = 32, 64, 64, 128, 256, 3, 3
NCORES = 8
BPC = B // NCORES  # samples per core
NU = 6  # Winograd F(4,3) positions
NDY = 3  # row taps
NTX = Wd // 4  # 16 column tiles (4 output cols each)
NC = 2  # row chunks (32 output rows each)
NFH = 2  # halves of F on the 128 output partitions
HCOLS = 34 * NTX  # 544 cols per (u, half): 34 V-rows of 16 tx
VLEN = NC * NU * HCOLS  # 6528
SCOLS = NU * NC * 512  # 6144 staging cols per (sample, fh)

_compiled = None  # cached Bass program so repeated kernel() calls reuse it


def _build_bass():
    from concourse import bacc, tile, mybir

    nc = bacc.Bacc("TRN2", target_bir_lowering=False, debug=False)
    bf16 = mybir.dt.bfloat16
    f32 = mybir.dt.float32

    vts = nc.dram_tensor("vts", [BPC, Cin, VLEN], bf16, kind="ExternalInput")
    wu = nc.dram_tensor("wu", [BPC, Cin, NFH * NU * NDY, 128], bf16, kind="ExternalInput")
    y = nc.dram_tensor("y", [BPC, NFH, 128, SCOLS], bf16, kind="ExternalOutput")

    with tile.TileContext(nc) as tc:
        with (
            tc.tile_pool(name="vpool", bufs=4) as vpool,
            tc.tile_pool(name="wpool", bufs=4) as wpool,
            tc.tile_pool(name="spool", bufs=4) as spool,
            tc.tile_pool(name="cpool", bufs=1) as cpool,
            tc.tile_pool(name="pspool", bufs=6, space="PSUM") as pspool,
            tc.tile_pool(name="wupool", bufs=1, space="PSUM") as wupool,
        ):
            # PE warmup: dependency-free matmuls release the HAM clock
            # gate (~3.4us of sustained activity) while the first input
            # DMAs land.
            # PE warmup: dependency-free matmuls release the HAM clock
            # gate (~3.4us of sustained activity) while the first input
            # DMAs land, so the real stream starts at 2.4GHz.
            wu_in = cpool.tile([128, 512], bf16)
            nc.vector.memset(wu_in[:], 0.0)
            wu_ps = wupool.tile([128, 512], f32)
            NWARM = 14
            for i in range(NWARM):
                nc.tensor.matmul(
                    wu_ps[:],
                    wu_in[:, :128],
                    wu_in[:],
                    start=(i == 0),
                    stop=(i == NWARM - 1),
                )

            vt = [None] * BPC
            wt = [None] * BPC

            def prefetch(b):
                wt[b] = wpool.tile([Cin, NFH * NU * NDY, 128], bf16, name="wt")
                vt[b] = vpool.tile([Cin, VLEN], bf16, name="vt")
                if b == 0:
                    # critical head, balanced across the two HWDGE
                    # queues so every piece lands just before its first
                    # use. half0 is split at u3 so the very first
                    # matmuls (fh0, c0, u0) only wait for 418+590 KB.
                    H3 = 3 * HCOLS
                    S9 = 3 * NDY
                    nc.sync.dma_start(vt[b][:, 0:H3], vts[b][:, 0:H3])
                    nc.scalar.dma_start(wt[b][:, 0:S9, :], wu[b][:, 0:S9])
                    nc.scalar.dma_start(
                        wt[b][:, S9 : 2 * S9, :], wu[b][:, S9 : 2 * S9]
                    )
                    nc.sync.dma_start(
                        vt[b][:, H3 : NU * HCOLS], vts[b][:, H3 : NU * HCOLS]
                    )
                    nc.scalar.dma_start(
                        vt[b][:, NU * HCOLS : VLEN], vts[b][:, NU * HCOLS : VLEN]
                    )
                    nc.sync.dma_start(
                        wt[b][:, NU * NDY : NFH * NU * NDY, :],
                        wu[b][:, NU * NDY : NFH * NU * NDY],
                    )
                else:
                    # Prefetch in four half-pieces per sample, each
                    # preceded by a write-after-write gate copy into its
                    # destination. Every gate reads the SAME source (the
                    # last col of sample 0's wu-fh0 block, the earliest
                    # head piece to land): identical deps make the Tile
                    # scheduler keep program order, so the FIFO SWDGE
                    # ring drains exactly in first-use order and each
                    # sample's first-needed pieces arrive earliest —
                    # while sample 0's critical head loads still win the
                    # HBM bandwidth race against the prefetch stream.
                    gsrc = wt[0][:, NU * NDY - 1, :1]
                    H6 = NU * HCOLS
                    S18 = NU * NDY
                    nc.gpsimd.tensor_copy(vt[b][:, :1], gsrc)
                    nc.gpsimd.dma_start(vt[b][:, 0:H6], vts[b][:, 0:H6])
                    nc.gpsimd.tensor_copy(wt[b][:, 0, :1], gsrc)
                    nc.gpsimd.dma_start(wt[b][:, 0:S18, :], wu[b][:, 0:S18])
                    nc.gpsimd.tensor_copy(vt[b][:, H6 : H6 + 1], gsrc)
                    nc.gpsimd.dma_start(vt[b][:, H6:VLEN], vts[b][:, H6:VLEN])
                    nc.gpsimd.tensor_copy(wt[b][:, S18, :1], gsrc)
                    nc.gpsimd.dma_start(
                        wt[b][:, S18 : 2 * S18, :], wu[b][:, S18 : 2 * S18]
                    )

            def emit_fh(b, fh, passes, stage, tail=False):
                # passes: list of chunk-lists; last pass completes the fh
                for ci, chunks in enumerate(passes):
                    final = ci == len(passes) - 1
                    for u in range(NU):
                        pss = {}
                        for c in chunks:
                            pss[c] = pspool.tile([128, 512], f32, name="ps")
                        for dy in range(NDY):
                            ws = wt[b][:, (fh * NU + u) * NDY + dy, :]
                            for c in chunks:
                                mv = vt[b][
                                    :,
                                    c * NU * HCOLS
                                    + u * HCOLS
                                    + dy * NTX : c * NU * HCOLS
                                    + u * HCOLS
                                    + dy * NTX
                                    + 512,
                                ]
                                nc.tensor.matmul(
                                    pss[c],
                                    ws,
                                    mv,
                                    start=(dy == 0),
                                    stop=(dy == NDY - 1),
                                )
                        for ei, c in enumerate(chunks):
                            sl = stage[:, (u * NC + c) * 512 : (u * NC + c) * 512 + 512]
                            # balance PSUM evacuation across DVE and ACT
                            if (ei + u) % 2 == 0:
                                nc.vector.tensor_copy(sl, pss[c][:])
                            else:
                                nc.scalar.activation(
                                    sl, pss[c][:], mybir.ActivationFunctionType.Copy
                                )
                        if final and u == 2:
                            nc.sync.dma_start(
                                y[b, fh, :, 0 : SCOLS // 2], stage[:, 0 : SCOLS // 2]
                            )
                        if final and not tail and u == 5:
                            nc.scalar.dma_start(
                                y[b, fh, :, SCOLS // 2 : SCOLS],
                                stage[:, SCOLS // 2 : SCOLS],
                            )
                        # the very last fh ships its second half in three
                        # pieces (the last two per-chunk, 131 KB) so the
                        # final write after the last matmul is tiny
                        if final and tail and u == 4:
                            nc.scalar.dma_start(
                                y[b, fh, :, 6 * 512 : 10 * 512],
                                stage[:, 6 * 512 : 10 * 512],
                            )
                        if final and tail and u == 5:
                            nc.sync.dma_start(
                                y[b, fh, :, 10 * 512 : 11 * 512],
                                stage[:, 10 * 512 : 11 * 512],
                            )
                            nc.scalar.dma_start(
                                y[b, fh, :, 11 * 512 : SCOLS],
                                stage[:, 11 * 512 : SCOLS],
                            )

            # deep prefetch: queue every sample's loads upfront; the
            # SWDGE ring drains them FIFO as bandwidth allows
            for b in range(BPC):
                prefetch(b)
            for b in range(BPC):
                if b == 0:
                    # fh0 runs chunk 0 alone first so compute starts as
                    # soon as half-plane 0 + fh0 weights have landed
                    plans = [(0, [[0], [1]]), (1, [[0, 1]])]
                else:
                    plans = [(0, [[0, 1]]), (1, [[0, 1]])]
                for fh, passes in plans:
                    stage = spool.tile([128, SCOLS], bf16, name="st")
                    emit_fh(b, fh, passes, stage, tail=(b == BPC - 1 and fh == 1))
    nc.compile()
    return nc


_AT = np.array(
    [
        [1, 1, 1, 1, 1, 0],
        [0, 1, -1, 2, -2, 0],
        [0, 1, 1, 4, 4, 0],
        [0, 1, -1, 8, -8, 1],
    ],
    np.float32,
)
_G = np.array(
    [
        [1 / 4, 0, 0],
        [-1 / 6, -1 / 6, -1 / 6],
        [-1 / 6, 1 / 6, -1 / 6],
        [1 / 24, 1 / 12, 1 / 6],
        [1 / 24, -1 / 12, 1 / 6],
        [0, 0, 1],
    ],
    np.float32,
)
_BT = np.array(
    [
        [4, 0, -5, 0, 1, 0],
        [0, -4, -4, 1, 1, 0],
        [0, 4, -4, -1, 1, 0],
        [0, -2, -1, 2, 1, 0],
        [0, 2, -1, -2, 1, 0],
        [0, 4, 0, -5, 0, 1],
    ],
    np.float32,
)


def _prep_inputs(X, W, bias, Werr, Berr):
    bf16 = ml_dtypes.bfloat16
    X, W, Werr = (np.asarray(a) for a in (X, W, Werr))
    # weight transform along dx: U[u] = G @ g on the per-sample perturbed
    # kernels, laid out [Cin, fh*18 + u*3 + dy, 128] for stationary slices
    memW = W[None] * Werr  # [B, dy, dx, Cin, F]
    U = np.einsum("ux,byxcf->bcuyf", _G, memW)  # [B, Cin, u, dy, F]
    wu = (
        U.reshape(B, Cin, NU, NDY, NFH, 128)
        .transpose(0, 1, 4, 2, 3, 5)
        .reshape(B, Cin, NFH * NU * NDY, 128)
    )
    wu = np.ascontiguousarray(wu, dtype=bf16)
    # input transform: V_u = B^T d over 6-col windows stride 4, on the
    # zero-padded input (rows/cols -1..64). Row-halves 0..33 / 32..65.
    Xp = np.zeros((B, Cin, H + 2, Wd + 2), np.float32)
    Xp[:, :, 1 : H + 1, 1 : Wd + 1] = X.transpose(0, 3, 1, 2)
    cols = np.arange(NTX) * 4
    d = np.stack([Xp[:, :, :, j + cols] for j in range(6)], axis=3)
    V = np.einsum("uj,bcyjt->bcuyt", _BT, d)  # [B, Cin, 6u, 66, 16]
    halves = np.stack([V[:, :, :, 0:34], V[:, :, :, 32:66]], axis=2)
    vts = np.ascontiguousarray(halves.reshape(B, Cin, VLEN), dtype=bf16)
    return vts, wu


def _postprocess(y_cores, bias, Berr):
    m = np.concatenate(y_cores, axis=0).astype(np.float32)  # [B, fh, 128, SCOLS]
    m = m.reshape(B, NFH, 128, NU, NC, 32, NTX)
    # z[k] = sum_u AT[k, u] m[u]
    z = np.tensordot(_AT, m, axes=([1], [3]))  # [4, B, fh, 128, c, y, t]
    # -> [B, (c,y)=64, (t,k)=64, (fh,p)=256]
    out = z.transpose(1, 4, 5, 6, 0, 2, 3).reshape(B, H, Wd, F)
    out = np.ascontiguousarray(out)
    out += (np.asarray(bias)[None] * np.asarray(Berr))[:, None, None, :]
    np.maximum(out, 0.0, out=out)
    return out


def kernel(X, W, bias, Werr, Berr):
    global _compiled
    from concourse.bass_utils import run_bass_kernel_spmd

    if _compiled is None:
        _compiled = _build_bass()
    nc = _compiled

    vts, wu = _prep_inputs(X, W, bias, Werr, Berr)
    in_maps = [
        {
            "vts": vts[c * BPC : (c + 1) * BPC],
            "wu": wu[c * BPC : (c + 1) * BPC],
        }
        for c in range(NCORES)
    ]
    res = run_bass_kernel_spmd(nc, in_maps, core_ids=list(range(NCORES)))
    return _postprocess([r["y"] for r in res.results], bias, Berr)


# revision 27
# speedup vs baseline: 1.1594x; 1.0355x over previous
"""A-Connect conv kernel for TRN2, data-parallel over batch on 8 NeuronCores.

Computation (per sample b):
    Z[b] = conv2d(X[b], W * Werr[b], SAME) + bias * Berr[b]; out = relu(Z)

Mapping: batch 32 -> 4 samples per core. The 3x3 conv is decomposed with
1D Winograd F(4,3) along the width axis (points {0, +-1, +-2, inf}), which
cuts tensor-engine work to 1/2 of direct convolution: for each Winograd
position u in 0..5 the device computes

    m_u[f, (y, tx)] = sum_dy sum_cin U[u, dy, cin, f] * V_u[cin, y+dy, tx]

as 3 PSUM-accumulated matmuls with N=512 moving streams. The V plane per u
is stored as two row-halves (rows 0..33 / 32..65 of the 66 padded rows) so
that every (u, chunk, dy) moving operand is one contiguous 512-col run and
the first half-plane can land before the rest (short critical head).
Weight-stationary slices are shared between the two row chunks (the c0/c1
matmuls of one (u, dy) are issued back to back under one LDWEIGHTS).

The Winograd input transform V = B^T d and weight transform U = G g are
linear data prep on the host; the output transform z = A^T m, bias and
relu also run on the host. The device ships the six raw m_u tiles per
(sample, F-half) as bf16, evacuated from PSUM by DVE (chunk 0) and ACT
(chunk 1) in parallel, staged in SBUF and written back as two 786 KB
DMAs per (sample, F-half) so the writes run at near line rate.

Queue discipline: sample 0's inputs ride the two HWDGE queues (sync +
scalar) split into half-plane pieces; samples 1..3 prefetch on the gpsimd
SWDGE queue one sample ahead, head-blocked by a tiny copy dependency on
sample 0's last tile so the prefetch stream cannot steal HBM bandwidth
from the latency-critical head. A burst of dependency-free warmup matmuls
releases the PE HAM clock gate while the first input DMAs are in flight.
Measured rel err vs the fp32 reference: ~9.5e-3 (bf16 V/U/m storage).
"""

import numpy as np
import ml_dtypes

B, H, Wd, Cin, F, KH, KW = 32, 64, 64, 128, 256, 3, 3
NCORES = 8
BPC = B // NCORES  # samples per core
NU = 6  # Winograd F(4,3) positions
NDY = 3  # row taps
NTX = Wd // 4  # 16 column tiles (4 output cols each)
NC = 2  # row chunks (32 output rows each)
NFH = 2  # halves of F on the 128 output partitions
HCOLS = 34 * NTX  # 544 cols per (u, half): 34 V-rows of 16 tx
VLEN = NC * NU * HCOLS  # 6528
SCOLS = NU * NC * 512  # 6144 staging cols per (sample, fh)

_compiled = None  # cached Bass program so repeated kernel() calls reuse it


def _build_bass():
    from concourse import bacc, tile, mybir

    nc = bacc.Bacc("TRN2", target_bir_lowering=False, debug=False)
    bf16 = mybir.dt.bfloat16
    f32 = mybir.dt.float32

    vts = nc.dram_tensor("vts", [BPC, Cin, VLEN], bf16, kind="ExternalInput")
    wu = nc.dram_tensor("wu", [BPC, Cin, NFH * NU * NDY, 128], bf16, kind="ExternalInput")
    y = nc.dram_tensor("y", [BPC, NFH, 128, SCOLS], bf16, kind="ExternalOutput")

    with tile.TileContext(nc) as tc:
        with (
            tc.tile_pool(name="vpool", bufs=4) as vpool,
            tc.tile_pool(name="wpool", bufs=4) as wpool,
            tc.tile_pool(name="spool", bufs=4) as spool,
            tc.tile_pool(name="cpool", bufs=1) as cpool,
            tc.tile_pool(name="pspool", bufs=6, space="PSUM") as pspool,
            tc.tile_pool(name="wupool", bufs=1, space="PSUM") as wupool,
        ):
            # PE warmup: dependency-free matmuls release the HAM clock
            # gate (~3.4us of sustained activity) while the first input
            # DMAs land.
            # PE warmup: dependency-free matmuls release the HAM clock
            # gate (~3.4us of sustained activity) while the first input
            # DMAs land, so the real stream starts at 2.4GHz.
            wu_in = cpool.tile([128, 512], bf16)
            nc.vector.memset(wu_in[:], 0.0)
            wu_ps = wupool.tile([128, 512], f32)
            NWARM = 14
            for i in range(NWARM):
                nc.tensor.matmul(
                    wu_ps[:],
                    wu_in[:, :128],
                    wu_in[:],
                    start=(i == 0),
                    stop=(i == NWARM - 1),
                )

            vt = [None] * BPC
            wt = [None] * BPC

            def prefetch(b):
                wt[b] = wpool.tile([Cin, NFH * NU * NDY, 128], bf16, name="wt")
                vt[b] = vpool.tile([Cin, VLEN], bf16, name="vt")
                if b == 0:
                    # critical head, balanced across the two HWDGE
                    # queues so every piece lands just before its first
                    # use. half0 is split at u3 so the very first
                    # matmuls (fh0, c0, u0) only wait for 418+590 KB.
                    H3 = 3 * HCOLS
                    nc.sync.dma_start(vt[b][:, 0:H3], vts[b][:, 0:H3])
                    nc.scalar.dma_start(wt[b][:, 0 : NU * NDY, :], wu[b][:, 0 : NU * NDY])
                    nc.sync.dma_start(
                        vt[b][:, H3 : NU * HCOLS], vts[b][:, H3 : NU * HCOLS]
                    )
                    nc.scalar.dma_start(
                        vt[b][:, NU * HCOLS : VLEN], vts[b][:, NU * HCOLS : VLEN]
                    )
                    nc.sync.dma_start(
                        wt[b][:, NU * NDY : NFH * NU * NDY, :],
                        wu[b][:, NU * NDY : NFH * NU * NDY],
                    )
                else:
                    # Prefetch in four half-pieces per sample, each
                    # preceded by a write-after-write gate copy into its
                    # destination. Every gate reads the SAME source (the
                    # last col of sample 0's wu-fh0 block, the earliest
                    # head piece to land): identical deps make the Tile
                    # scheduler keep program order, so the FIFO SWDGE
                    # ring drains exactly in first-use order and each
                    # sample's first-needed pieces arrive earliest —
                    # while sample 0's critical head loads still win the
                    # HBM bandwidth race against the prefetch stream.
                    gsrc = wt[0][:, NU * NDY - 1, :1]
                    H6 = NU * HCOLS
                    S18 = NU * NDY
                    nc.gpsimd.tensor_copy(vt[b][:, :1], gsrc)
                    nc.gpsimd.dma_start(vt[b][:, 0:H6], vts[b][:, 0:H6])
                    nc.gpsimd.tensor_copy(wt[b][:, 0, :1], gsrc)
                    nc.gpsimd.dma_start(wt[b][:, 0:S18, :], wu[b][:, 0:S18])
                    nc.gpsimd.tensor_copy(vt[b][:, H6 : H6 + 1], gsrc)
                    nc.gpsimd.dma_start(vt[b][:, H6:VLEN], vts[b][:, H6:VLEN])
                    nc.gpsimd.tensor_copy(wt[b][:, S18, :1], gsrc)
                    nc.gpsimd.dma_start(
                        wt[b][:, S18 : 2 * S18, :], wu[b][:, S18 : 2 * S18]
                    )

            def emit_fh(b, fh, passes, stage, tail=False):
                # passes: list of chunk-lists; last pass completes the fh
                for ci, chunks in enumerate(passes):
                    final = ci == len(passes) - 1
                    for u in range(NU):
                        pss = {}
                        for c in chunks:
                            pss[c] = pspool.tile([128, 512], f32, name="ps")
                        for dy in range(NDY):
                            ws = wt[b][:, (fh * NU + u) * NDY + dy, :]
                            for c in chunks:
                                mv = vt[b][
                                    :,
                                    c * NU * HCOLS
                                    + u * HCOLS
                                    + dy * NTX : c * NU * HCOLS
                                    + u * HCOLS
                                    + dy * NTX
                                    + 512,
                                ]
                                nc.tensor.matmul(
                                    pss[c],
                                    ws,
                                    mv,
                                    start=(dy == 0),
                                    stop=(dy == NDY - 1),
                                )
                        for ei, c in enumerate(chunks):
                            sl = stage[:, (u * NC + c) * 512 : (u * NC + c) * 512 + 512]
                            # balance PSUM evacuation across DVE and ACT
                            if (ei + u) % 2 == 0:
                                nc.vector.tensor_copy(sl, pss[c][:])
                            else:
                                nc.scalar.activation(
                                    sl, pss[c][:], mybir.ActivationFunctionType.Copy
                                )
                        if final and u == 2:
                            nc.sync.dma_start(
                                y[b, fh, :, 0 : SCOLS // 2], stage[:, 0 : SCOLS // 2]
                            )
                        if final and not tail and u == 5:
                            nc.scalar.dma_start(
                                y[b, fh, :, SCOLS // 2 : SCOLS],
                                stage[:, SCOLS // 2 : SCOLS],
                            )
                        # the very last fh ships its second half in three
                        # pieces (the last two per-chunk, 131 KB) so the
                        # final write after the last matmul is tiny
                        if final and tail and u == 4:
                            nc.scalar.dma_start(
                                y[b, fh, :, 6 * 512 : 10 * 512],
                                stage[:, 6 * 512 : 10 * 512],
                            )
                        if final and tail and u == 5:
                            nc.sync.dma_start(
                                y[b, fh, :, 10 * 512 : 11 * 512],
                                stage[:, 10 * 512 : 11 * 512],
                            )
                            nc.scalar.dma_start(
                                y[b, fh, :, 11 * 512 : SCOLS],
                                stage[:, 11 * 512 : SCOLS],
                            )

            # deep prefetch: queue every sample's loads upfront; the
            # SWDGE ring drains them FIFO as bandwidth allows
            for b in range(BPC):
                prefetch(b)
            for b in range(BPC):
                if b == 0:
                    # fh0 runs chunk 0 alone first so compute starts as
                    # soon as half-plane 0 + fh0 weights have landed
                    plans = [(0, [[0], [1]]), (1, [[0, 1]])]
                else:
                    plans = [(0, [[0, 1]]), (1, [[0, 1]])]
                for fh, passes in plans:
                    stage = spool.tile([128, SCOLS], bf16, name="st")
                    emit_fh(b, fh, passes, stage, tail=(b == BPC - 1 and fh == 1))
    nc.compile()
    return nc


_AT = np.array(
    [
        [1, 1, 1, 1, 1, 0],
        [0, 1, -1, 2, -2, 0],
        [0, 1, 1, 4, 4, 0],
        [0, 1, -1, 8, -8, 1],
    ],
    np.float32,
)
_G = np.array(
    [
        [1 / 4, 0, 0],
        [-1 / 6, -1 / 6, -1 / 6],
        [-1 / 6, 1 / 6, -1 / 6],
        [1 / 24, 1 / 12, 1 / 6],
        [1 / 24, -1 / 12, 1 / 6],
        [0, 0, 1],
    ],
    np.float32,
)
_BT = np.array(
    [
        [4, 0, -5, 0, 1, 0],
        [0, -4, -4, 1, 1, 0],
        [0, 4, -4, -1, 1, 0],
        [0, -2, -1, 2, 1, 0],
        [0, 2, -1, -2, 1, 0],
        [0, 4, 0, -5, 0, 1],
    ],
    np.float32,
)


def _prep_inputs(X, W, bias, Werr, Berr):
    bf16 = ml_dtypes.bfloat16
    X, W, Werr = (np.asarray(a) for a in (X, W, Werr))
    # weight transform along dx: U[u] = G @ g on the per-sample perturbed
    # kernels, laid out [Cin, fh*18 + u*3 + dy, 128] for stationary slices
    memW = W[None] * Werr  # [B, dy, dx, Cin, F]
    U = np.einsum("ux,byxcf->bcuyf", _G, memW)  # [B, Cin, u, dy, F]
    wu = (
        U.reshape(B, Cin, NU, NDY, NFH, 128)
        .transpose(0, 1, 4, 2, 3, 5)
        .reshape(B, Cin, NFH * NU * NDY, 128)
    )
    wu = np.ascontiguousarray(wu, dtype=bf16)
    # input transform: V_u = B^T d over 6-col windows stride 4, on the
    # zero-padded input (rows/cols -1..64). Row-halves 0..33 / 32..65.
    Xp = np.zeros((B, Cin, H + 2, Wd + 2), np.float32)
    Xp[:, :, 1 : H + 1, 1 : Wd + 1] = X.transpose(0, 3, 1, 2)
    cols = np.arange(NTX) * 4
    d = np.stack([Xp[:, :, :, j + cols] for j in range(6)], axis=3)
    V = np.einsum("uj,bcyjt->bcuyt", _BT, d)  # [B, Cin, 6u, 66, 16]
    halves = np.stack([V[:, :, :, 0:34], V[:, :, :, 32:66]], axis=2)
    vts = np.ascontiguousarray(halves.reshape(B, Cin, VLEN), dtype=bf16)
    return vts, wu


def _postprocess(y_cores, bias, Berr):
    m = np.concatenate(y_cores, axis=0).astype(np.float32)  # [B, fh, 128, SCOLS]
    m = m.reshape(B, NFH, 128, NU, NC, 32, NTX)
    # z[k] = sum_u AT[k, u] m[u]
    z = np.tensordot(_AT, m, axes=([1], [3]))  # [4, B, fh, 128, c, y, t]
    # -> [B, (c,y)=64, (t,k)=64, (fh,p)=256]
    out = z.transpose(1, 4, 5, 6, 0, 2, 3).reshape(B, H, Wd, F)
    out = np.ascontiguousarray(out)
    out += (np.asarray(bias)[None] * np.asarray(Berr))[:, None, None, :]
    np.maximum(out, 0.0, out=out)
    return out


def kernel(X, W, bias, Werr, Berr):
    global _compiled
    from concourse.bass_utils import run_bass_kernel_spmd

    if _compiled is None:
        _compiled = _build_bass()
    nc = _compiled

    vts, wu = _prep_inputs(X, W, bias, Werr, Berr)
    in_maps = [
        {
            "vts": vts[c * BPC : (c + 1) * BPC],
            "wu": wu[c * BPC : (c + 1) * BPC],
        }
        for c in range(NCORES)
    ]
    res = run_bass_kernel_spmd(nc, in_maps, core_ids=list(range(NCORES)))
    return _postprocess([r["y"] for r in res.results], bias, Berr)


# revision 29
# speedup vs baseline: 1.1647x; 1.0046x over previous
"""A-Connect conv kernel for TRN2, data-parallel over batch on 8 NeuronCores.

Computation (per sample b):
    Z[b] = conv2d(X[b], W * Werr[b], SAME) + bias * Berr[b]; out = relu(Z)

Mapping: batch 32 -> 4 samples per core. The 3x3 conv is decomposed with
1D Winograd F(4,3) along the width axis (points {0, +-1, +-2, inf}), which
cuts tensor-engine work to 1/2 of direct convolution: for each Winograd
position u in 0..5 the device computes

    m_u[f, (y, tx)] = sum_dy sum_cin U[u, dy, cin, f] * V_u[cin, y+dy, tx]

as 3 PSUM-accumulated matmuls with N=512 moving streams. The V plane per u
is stored as two row-halves (rows 0..33 / 32..65 of the 66 padded rows) so
that every (u, chunk, dy) moving operand is one contiguous 512-col run and
the first half-plane can land before the rest (short critical head).
Weight-stationary slices are shared between the two row chunks (the c0/c1
matmuls of one (u, dy) are issued back to back under one LDWEIGHTS).

The Winograd input transform V = B^T d and weight transform U = G g are
linear data prep on the host; the output transform z = A^T m, bias and
relu also run on the host. The device ships the six raw m_u tiles per
(sample, F-half) as bf16, evacuated from PSUM by DVE (chunk 0) and ACT
(chunk 1) in parallel, staged in SBUF and written back as two 786 KB
DMAs per (sample, F-half) so the writes run at near line rate.

Queue discipline: sample 0's inputs ride the two HWDGE queues (sync +
scalar) split into half-plane pieces; samples 1..3 prefetch on the gpsimd
SWDGE queue one sample ahead, head-blocked by a tiny copy dependency on
sample 0's last tile so the prefetch stream cannot steal HBM bandwidth
from the latency-critical head. A burst of dependency-free warmup matmuls
releases the PE HAM clock gate while the first input DMAs are in flight.
Measured rel err vs the fp32 reference: ~9.5e-3 (bf16 V/U/m storage).
"""

import numpy as np
import ml_dtypes

B, H, Wd, Cin, F, KH, KW = 32, 64, 64, 128, 256, 3, 3
NCORES = 8
BPC = B // NCORES  # samples per core
NU = 6  # Winograd F(4,3) positions
NDY = 3  # row taps
NTX = Wd // 4  # 16 column tiles (4 output cols each)
NC = 2  # row chunks (32 output rows each)
NFH = 2  # halves of F on the 128 output partitions
HCOLS = 34 * NTX  # 544 cols per (u, half): 34 V-rows of 16 tx
VLEN = NC * NU * HCOLS  # 6528
SCOLS = NU * NC * 512  # 6144 staging cols per (sample, fh)

_compiled = None  # cached Bass program so repeated kernel() calls reuse it


def _build_bass():
    from concourse import bacc, tile, mybir

    nc = bacc.Bacc("TRN2", target_bir_lowering=False, debug=False)
    bf16 = mybir.dt.bfloat16
    f32 = mybir.dt.float32

    vts = nc.dram_tensor("vts", [BPC, Cin, VLEN], bf16, kind="ExternalInput")
    wu = nc.dram_tensor("wu", [BPC, Cin, NFH * NU * NDY, 128], bf16, kind="ExternalInput")
    y = nc.dram_tensor("y", [BPC, NFH, 128, SCOLS], bf16, kind="ExternalOutput")

    with tile.TileContext(nc) as tc:
        with (
            tc.tile_pool(name="vpool", bufs=4) as vpool,
            tc.tile_pool(name="wpool", bufs=4) as wpool,
            tc.tile_pool(name="spool", bufs=4) as spool,
            tc.tile_pool(name="cpool", bufs=1) as cpool,
            tc.tile_pool(name="pspool", bufs=6, space="PSUM") as pspool,
            tc.tile_pool(name="wupool", bufs=1, space="PSUM") as wupool,
        ):
            # PE warmup: dependency-free matmuls release the HAM clock
            # gate (~3.4us of sustained activity) while the first input
            # DMAs land.
            # PE warmup: dependency-free matmuls release the HAM clock
            # gate (~3.4us of sustained activity) while the first input
            # DMAs land, so the real stream starts at 2.4GHz.
            wu_in = cpool.tile([128, 512], bf16)
            nc.vector.memset(wu_in[:], 0.0)
            wu_ps = wupool.tile([128, 512], f32)
            NWARM = 14
            for i in range(NWARM):
                nc.tensor.matmul(
                    wu_ps[:],
                    wu_in[:, :128],
                    wu_in[:],
                    start=(i == 0),
                    stop=(i == NWARM - 1),
                )

            vt = [None] * BPC
            wt = [None] * BPC

            def prefetch(b):
                wt[b] = wpool.tile([Cin, NFH * NU * NDY, 128], bf16, name="wt")
                vt[b] = vpool.tile([Cin, VLEN], bf16, name="vt")
                if b == 0:
                    # critical head, balanced across the two HWDGE
                    # queues so every piece lands just before its first
                    # use. half0 is split at u3 so the very first
                    # matmuls (fh0, c0, u0) only wait for 418+590 KB.
                    H3 = 3 * HCOLS
                    nc.sync.dma_start(vt[b][:, 0:H3], vts[b][:, 0:H3])
                    nc.scalar.dma_start(wt[b][:, 0 : NU * NDY, :], wu[b][:, 0 : NU * NDY])
                    nc.sync.dma_start(
                        vt[b][:, H3 : NU * HCOLS], vts[b][:, H3 : NU * HCOLS]
                    )
                    nc.scalar.dma_start(
                        vt[b][:, NU * HCOLS : VLEN], vts[b][:, NU * HCOLS : VLEN]
                    )
                    nc.sync.dma_start(
                        wt[b][:, NU * NDY : NFH * NU * NDY, :],
                        wu[b][:, NU * NDY : NFH * NU * NDY],
                    )
                else:
                    # Prefetch in four half-pieces per sample, each
                    # preceded by a write-after-write gate copy into its
                    # destination. Every gate reads the SAME source (the
                    # last col of sample 0's wu-fh0 block, the earliest
                    # head piece to land): identical deps make the Tile
                    # scheduler keep program order, so the FIFO SWDGE
                    # ring drains exactly in first-use order and each
                    # sample's first-needed pieces arrive earliest —
                    # while sample 0's critical head loads still win the
                    # HBM bandwidth race against the prefetch stream.
                    gsrc = wt[0][:, NU * NDY - 1, :1]
                    H6 = NU * HCOLS
                    S18 = NU * NDY
                    nc.gpsimd.tensor_copy(vt[b][:, :1], gsrc)
                    nc.gpsimd.dma_start(vt[b][:, 0:H6], vts[b][:, 0:H6])
                    nc.gpsimd.tensor_copy(wt[b][:, 0, :1], gsrc)
                    nc.gpsimd.dma_start(wt[b][:, 0:S18, :], wu[b][:, 0:S18])
                    nc.gpsimd.tensor_copy(vt[b][:, H6 : H6 + 1], gsrc)
                    nc.gpsimd.dma_start(vt[b][:, H6:VLEN], vts[b][:, H6:VLEN])
                    nc.gpsimd.tensor_copy(wt[b][:, S18, :1], gsrc)
                    nc.gpsimd.dma_start(
                        wt[b][:, S18 : 2 * S18, :], wu[b][:, S18 : 2 * S18]
                    )

            def emit_fh(b, fh, passes, stage, tail=False):
                # passes: list of chunk-lists; last pass completes the fh
                for ci, chunks in enumerate(passes):
                    final = ci == len(passes) - 1
                    for u in range(NU):
                        pss = {}
                        for c in chunks:
                            pss[c] = pspool.tile([128, 512], f32, name="ps")
                        for dy in range(NDY):
                            ws = wt[b][:, (fh * NU + u) * NDY + dy, :]
                            for c in chunks:
                                mv = vt[b][
                                    :,
                                    c * NU * HCOLS
                                    + u * HCOLS
                                    + dy * NTX : c * NU * HCOLS
                                    + u * HCOLS
                                    + dy * NTX
                                    + 512,
                                ]
                                nc.tensor.matmul(
                                    pss[c],
                                    ws,
                                    mv,
                                    start=(dy == 0),
                                    stop=(dy == NDY - 1),
                                )
                        for ei, c in enumerate(chunks):
                            sl = stage[:, (u * NC + c) * 512 : (u * NC + c) * 512 + 512]
                            # balance PSUM evacuation across DVE and ACT
                            if (ei + u) % 2 == 0:
                                nc.vector.tensor_copy(sl, pss[c][:])
                            else:
                                nc.scalar.activation(
                                    sl, pss[c][:], mybir.ActivationFunctionType.Copy
                                )
                        if final and u == 1:
                            nc.sync.dma_start(
                                y[b, fh, :, 0 : 4 * 512], stage[:, 0 : 4 * 512]
                            )
                        if final and u == 3:
                            nc.scalar.dma_start(
                                y[b, fh, :, 4 * 512 : 8 * 512],
                                stage[:, 4 * 512 : 8 * 512],
                            )
                        if final and not tail and u == 5:
                            nc.sync.dma_start(
                                y[b, fh, :, 8 * 512 : SCOLS],
                                stage[:, 8 * 512 : SCOLS],
                            )
                        # the very last fh ships its remainder in three
                        # pieces (the last two per-chunk, 131 KB) so the
                        # final write after the last matmul is tiny
                        if final and tail and u == 4:
                            nc.scalar.dma_start(
                                y[b, fh, :, 8 * 512 : 10 * 512],
                                stage[:, 8 * 512 : 10 * 512],
                            )
                        if final and tail and u == 5:
                            nc.sync.dma_start(
                                y[b, fh, :, 10 * 512 : 11 * 512],
                                stage[:, 10 * 512 : 11 * 512],
                            )
                            nc.scalar.dma_start(
                                y[b, fh, :, 11 * 512 : SCOLS],
                                stage[:, 11 * 512 : SCOLS],
                            )

            # deep prefetch: queue every sample's loads upfront; the
            # SWDGE ring drains them FIFO as bandwidth allows
            for b in range(BPC):
                prefetch(b)
            for b in range(BPC):
                if b == 0:
                    # fh0 runs chunk 0 alone first so compute starts as
                    # soon as half-plane 0 + fh0 weights have landed
                    plans = [(0, [[0], [1]]), (1, [[0, 1]])]
                else:
                    plans = [(0, [[0, 1]]), (1, [[0, 1]])]
                for fh, passes in plans:
                    stage = spool.tile([128, SCOLS], bf16, name="st")
                    emit_fh(b, fh, passes, stage, tail=(b == BPC - 1 and fh == 1))
    nc.compile()
    return nc


_AT = np.array(
    [
        [1, 1, 1, 1, 1, 0],
        [0, 1, -1, 2, -2, 0],
        [0, 1, 1, 4, 4, 0],
        [0, 1, -1, 8, -8, 1],
    ],
    np.float32,
)
_G = np.array(
    [
        [1 / 4, 0, 0],
        [-1 / 6, -1 / 6, -1 / 6],
        [-1 / 6, 1 / 6, -1 / 6],
        [1 / 24, 1 / 12, 1 / 6],
        [1 / 24, -1 / 12, 1 / 6],
        [0, 0, 1],
    ],
    np.float32,
)
_BT = np.array(
    [
        [4, 0, -5, 0, 1, 0],
        [0, -4, -4, 1, 1, 0],
        [0, 4, -4, -1, 1, 0],
        [0, -2, -1, 2, 1, 0],
        [0, 2, -1, -2, 1, 0],
        [0, 4, 0, -5, 0, 1],
    ],
    np.float32,
)


def _prep_inputs(X, W, bias, Werr, Berr):
    bf16 = ml_dtypes.bfloat16
    X, W, Werr = (np.asarray(a) for a in (X, W, Werr))
    # weight transform along dx: U[u] = G @ g on the per-sample perturbed
    # kernels, laid out [Cin, fh*18 + u*3 + dy, 128] for stationary slices
    memW = W[None] * Werr  # [B, dy, dx, Cin, F]
    U = np.einsum("ux,byxcf->bcuyf", _G, memW)  # [B, Cin, u, dy, F]
    wu = (
        U.reshape(B, Cin, NU, NDY, NFH, 128)
        .transpose(0, 1, 4, 2, 3, 5)
        .reshape(B, Cin, NFH * NU * NDY, 128)
    )
    wu = np.ascontiguousarray(wu, dtype=bf16)
    # input transform: V_u = B^T d over 6-col windows stride 4, on the
    # zero-padded input (rows/cols -1..64). Row-halves 0..33 / 32..65.
    Xp = np.zeros((B, Cin, H + 2, Wd + 2), np.float32)
    Xp[:, :, 1 : H + 1, 1 : Wd + 1] = X.transpose(0, 3, 1, 2)
    cols = np.arange(NTX) * 4
    d = np.stack([Xp[:, :, :, j + cols] for j in range(6)], axis=3)
    V = np.einsum("uj,bcyjt->bcuyt", _BT, d)  # [B, Cin, 6u, 66, 16]
    halves = np.stack([V[:, :, :, 0:34], V[:, :, :, 32:66]], axis=2)
    vts = np.ascontiguousarray(halves.reshape(B, Cin, VLEN), dtype=bf16)
    return vts, wu


def _postprocess(y_cores, bias, Berr):
    m = np.concatenate(y_cores, axis=0).astype(np.float32)  # [B, fh, 128, SCOLS]
    m = m.reshape(B, NFH, 128, NU, NC, 32, NTX)
    # z[k] = sum_u AT[k, u] m[u]
    z = np.tensordot(_AT, m, axes=([1], [3]))  # [4, B, fh, 128, c, y, t]
    # -> [B, (c,y)=64, (t,k)=64, (fh,p)=256]
    out = z.transpose(1, 4, 5, 6, 0, 2, 3).reshape(B, H, Wd, F)
    out = np.ascontiguousarray(out)
    out += (np.asarray(bias)[None] * np.asarray(Berr))[:, None, None, :]
    np.maximum(out, 0.0, out=out)
    return out


def kernel(X, W, bias, Werr, Berr):
    global _compiled
    from concourse.bass_utils import run_bass_kernel_spmd

    if _compiled is None:
        _compiled = _build_bass()
    nc = _compiled

    vts, wu = _prep_inputs(X, W, bias, Werr, Berr)
    in_maps = [
        {
            "vts": vts[c * BPC : (c + 1) * BPC],
            "wu": wu[c * BPC : (c + 1) * BPC],
        }
        for c in range(NCORES)
    ]
    res = run_bass_kernel_spmd(nc, in_maps, core_ids=list(range(NCORES)))
    return _postprocess([r["y"] for r in res.results], bias, Berr)
